# revision 39
# baseline (speedup 1.0000x reference)
"""Trainium2 Bass kernel for nn_BinaryMemoryRNN (scatter_memory).

Computation (reference):
    logits = h_prev @ Mw.T + Mb                 # [B, 28]
    b1/b2  = bits of logits halves (> 0)
    idx1   = clip(sum(b1 * 2^(13-j)), 0, 8191)
    idx2   = clip(sum(b2 * 2^(13-j)), 8192, 16383)
    pre    = x @ Ww.T + h_prev @ Uw.T + mem[idx1] @ Qrw.T + mem[idx2] @ Qlw.T + b
    out    = sigmoid(layernorm(pre) * gamma + beta)

Strategy: data-parallel over batch across 8 cores (1024 rows each).
  - All four big matmuls in fp8 (e4m3) with DoubleRow (2x PE throughput).
    Weights scaled by 512 (LayerNorm is scale-invariant; bias scaled too).
  - Memory table stored centered (mem - 0.5) in fp8; the 0.5*rowsum(Q)
    correction is folded into the bias. Rows are fetched with
    gpsimd.dma_gather(transpose=True), which lands them 16-bit-granular
    interleaved in [feature, batch] layout; the byte-interleaved pairs are
    consumed directly by DoubleRowSwInterleave matmuls (no PE transposes).
    SWInterleave reads stationary columns reversed; this is absorbed by
    staging x8/h8 with each 128-batch block reversed on the host and
    un-reversing the output on the host.
  - logits via split-bf16 (h16@MwHi + h16@MwLo + r16@MwHi) — ~3x faster than
    the quarter-rate fp32 matmul, index-exact vs fp32.
  - The whole index pipeline (logits/bits/idx/wrap/gather) is split by batch
    half so the first gathers issue ~15us earlier.
  - Epilogue: bias add + bn_stats on DVE, rstd via quake-rsqrt bit trick on
    DVE (avoids ACT sqrt<->sigmoid table thrash), single fused
    scale+bias+Sigmoid on ACT, bf16 output.
  - PE HAM warmup matmuls at t0 so the index pipeline runs at 2.4 GHz.
"""

import sys

sys.path.insert(0, "/opt/trn_rl_repo")

from contextlib import ExitStack

import numpy as np
import ml_dtypes

import concourse.bass as bass
import concourse.tile as tile
from concourse import bacc, mybir, library_config
from concourse.bass_utils import run_bass_kernel_spmd

F32 = mybir.dt.float32
BF16 = mybir.dt.bfloat16
F8 = mybir.dt.float8e4
I16 = mybir.dt.int16
I32 = mybir.dt.int32
f8np = ml_dtypes.float8_e4m3fn
bfnp = ml_dtypes.bfloat16

B, I, H, NB = 8192, 1024, 1024, 14
MEM = 2**NB
NCORES = 8
BL = B // NCORES  # 1024 batch rows per core
HB = BL // 2  # 512-row batch half
KC = H // 128  # 8 contraction chunks of 128
MT = BL // 128  # 8 output row-tiles per core
EPS = 1e-5
WSCALE = 512.0
EPS_S = EPS * WSCALE * WSCALE
QUAKE = 0x5F3759DF

_CACHE = {}


def _build(trivial_gb: bool):
    nc = bacc.Bacc(
        "TRN2", target_bir_lowering=False, debug=False, enable_asserts=False
    )

    # activations: feat-major [128, KC, BL]; x8/h8 batch-reversed per 128-block
    x8_t = nc.dram_tensor("x8_t", [128, KC, BL], F8, kind="ExternalInput").ap()
    h8_t = nc.dram_tensor("h8_t", [128, KC, BL], F8, kind="ExternalInput").ap()
    # h split for exact-index logits: per batch-half bf16 high + residual
    h16_t = [
        nc.dram_tensor(f"h16{i}_t", [128, KC, HB], BF16, kind="ExternalInput").ap()
        for i in range(2)
    ]
    r16_t = [
        nc.dram_tensor(f"r16{i}_t", [128, KC, HB], BF16, kind="ExternalInput").ap()
        for i in range(2)
    ]
    # W/U weights: [2, 128, KC, H] fp8, w[s][p,k,n] = Ws[n, 128k+p]*S
    wxu_t = nc.dram_tensor("wxu_t", [2, 128, KC, H], F8, kind="ExternalInput").ap()
    # Qr/Ql weights: [2, 128, 4, 2, H] fp8, w[s][p,c,b,n] = Qs[n, 2*(128c+p)+b]*S
    wq_t = nc.dram_tensor("wq_t", [2, 128, 4, 2, H], F8, kind="ExternalInput").ap()
    # critical consts: mw-bf16-pairs[0:224] | clip[224:226] | negmb[226:227] |
    # pw-as-bf16[227:228]
    NCC = 228
    constc_t = nc.dram_tensor("constc_t", [128, NCC], F32, kind="ExternalInput").ap()
    bias_t = nc.dram_tensor("bias_t", [128, H], F32, kind="ExternalInput").ap()
    mem_t = nc.dram_tensor("mem_t", [MEM, H], F8, kind="ExternalInput").ap()
    if not trivial_gb:
        gam_t = nc.dram_tensor("gam_t", [128, H], F32, kind="ExternalInput").ap()
        bet_t = nc.dram_tensor("bet_t", [128, H], F32, kind="ExternalInput").ap()
    out_t = nc.dram_tensor("out_t", [BL, H], BF16, kind="ExternalOutput").ap()

    DR = mybir.MatmulPerfMode.DoubleRow
    DRI = mybir.MatmulPerfMode.DoubleRowSwInterleave

    with tile.TileContext(nc) as tc:
        with ExitStack() as ctx:
            cpool = ctx.enter_context(tc.tile_pool(name="consts", bufs=1))
            apool = ctx.enter_context(tc.tile_pool(name="acts", bufs=1))
            spool = ctx.enter_context(tc.tile_pool(name="small", bufs=2))
            epool = ctx.enter_context(tc.tile_pool(name="epilogue", bufs=2))
            pp_main = ctx.enter_context(
                tc.tile_pool(name="psum_main", bufs=3, space="PSUM")
            )
            pp_small = ctx.enter_context(
                tc.tile_pool(name="psum_small", bufs=2, space="PSUM")
            )

            nc.gpsimd.load_library(library_config.attnmlp)

            eps_sb = cpool.tile([128, 2], F32, tag="eps")
            nc.vector.memset(eps_sb[:, 0:1], EPS_S)
            nc.vector.memset(eps_sb[:, 1:2], 0.0)
            # prefetch the sigmoid table set while DMAs run
            dum_sb = cpool.tile([128, 1], F32, tag="dum")
            nc.vector.memset(dum_sb[:], 0.0)
            nc.scalar.activation(
                dum_sb[:], dum_sb[:], mybir.ActivationFunctionType.Sigmoid
            )

            # ---------------- input loads (one HWDGE queue, in priority order)
            constc_sb = cpool.tile([128, NCC], F32, tag="constc")
            nc.sync.dma_start(constc_sb[:], constc_t[:])
            mw_bf = constc_sb[:, 0:224].bitcast(BF16).rearrange(
                "p (k j t) -> p k j t", j=2 * NB, t=2
            )
            clip_sb = constc_sb[0:2, 224:226]
            negmb_sb = constc_sb[0 : 2 * NB, 226:227]
            pw_sb = constc_sb[0 : 2 * NB, 227:228].bitcast(BF16)

            # order: index-pipeline tensors first (h16a r16a h16b r16b), then
            # x8/wxu/h8 for the xh matmuls, then wq/bias for the mem phase
            h16_sb, r16_sb = [None, None], [None, None]
            for i in range(2):
                h16_sb[i] = spool.tile(
                    [128, KC, HB], BF16, tag=f"h16{i}", bufs=1, name=f"h16s{i}"
                )
                r16_sb[i] = spool.tile(
                    [128, KC, HB], BF16, tag=f"r16{i}", bufs=1, name=f"r16s{i}"
                )
            nc.sync.dma_start(h16_sb[0][:], h16_t[0][:])
            nc.sync.dma_start(r16_sb[0][:], r16_t[0][:])
            # x8/h8 split by batch half: tiles 0-3 only need columns 0:512,
            # so their first xh matmuls start ~2-3us earlier
            x8_sb = apool.tile([128, KC, BL], F8, tag="x8")
            nc.sync.dma_start(x8_sb[:, :, 0:HB], x8_t[:, :, 0:HB])
            wxu_sb = []
            for s in range(2):
                w = cpool.tile([128, KC, H], F8, tag=f"wxu{s}")
                nc.sync.dma_start(w[:], wxu_t[s])
                wxu_sb.append(w)
            h8_sb = apool.tile([128, KC, BL], F8, tag="h8")
            nc.sync.dma_start(h8_sb[:, :, 0:HB], h8_t[:, :, 0:HB])
            nc.sync.dma_start(h16_sb[1][:], h16_t[1][:])
            nc.sync.dma_start(r16_sb[1][:], r16_t[1][:])
            nc.sync.dma_start(x8_sb[:, :, HB:BL], x8_t[:, :, HB:BL])
            nc.sync.dma_start(h8_sb[:, :, HB:BL], h8_t[:, :, HB:BL])
            wq_sb = []
            for s in range(2):
                w = cpool.tile([128, 4, 2, H], F8, tag=f"wq{s}")
                nc.sync.dma_start(w[:], wq_t[s])
                wq_sb.append(w)
            bias_sb = cpool.tile([128, H], F32, tag="bias")
            nc.sync.dma_start(bias_sb[:], bias_t[:])
            if not trivial_gb:
                gam_sb = cpool.tile([128, H], F32, tag="gam")
                nc.sync.dma_start(gam_sb[:], gam_t[:])
                bet_sb = cpool.tile([128, H], F32, tag="bet")
                nc.sync.dma_start(bet_sb[:], bet_t[:])

            # ---------------- PE HAM warmup (junk matmuls on a memset tile,
            # independent of any DMA so they start right after the preamble)
            warm_sb = cpool.tile([128, 256], BF16, tag="warm")
            nc.vector.memset(warm_sb[:], 0.0)
            for w in range(32):
                wps = pp_main.tile([128, 512], F32, tag="acc", name=f"wm{w}")
                nc.tensor.matmul(
                    wps[:, 0:256], warm_sb[:, 0:128], warm_sb[:],
                    start=True, stop=True,
                )

            # ---------------- per-half index pipeline ----------------
            bits_sb = spool.tile([2 * NB, BL], BF16, tag="bits", bufs=1)
            # padded to 1536 so 512-span APs near the end stay in bounds
            idx16 = spool.tile([2, BL + HB], I16, tag="idx16", bufs=1)
            g_tiles = [[None] * 4, [None] * 4]  # [hf][k] -> [128, 4, 512]

            def emit_logits_half(hf):
                hsl = slice(hf * HB, (hf + 1) * HB)
                # logits.T [28, HB]: bf16 split — h16@MwHi + h16@MwLo + r16@MwHi
                lg = pp_small.tile([2 * NB, HB], F32, tag="sm", name=f"lg{hf}")
                first, last = (0, 0), (2, KC - 1)
                for g, (act, mt) in enumerate(
                    ((h16_sb[hf], 0), (h16_sb[hf], 1), (r16_sb[hf], 0))
                ):
                    for k in range(KC):
                        nc.tensor.matmul(
                            lg[:],
                            mw_bf[:, k, :, mt],
                            act[:, k, :],
                            start=((g, k) == first),
                            stop=((g, k) == last),
                        )
                nc.vector.tensor_scalar(
                    bits_sb[:, hsl], lg[:], negmb_sb[:, 0:1], None,
                    mybir.AluOpType.is_gt,
                )
                ix = pp_small.tile([2, HB], F32, tag="sm", name=f"ix{hf}")
                nc.tensor.matmul(
                    ix[:], pw_sb, bits_sb[:, hsl], start=True, stop=True
                )
                nc.vector.tensor_scalar(
                    idx16[:, hsl], ix[:], clip_sb[:, 0:1], clip_sb[:, 1:2],
                    mybir.AluOpType.max, mybir.AluOpType.min,
                )

            def emit_gather_half(hf):
                # Wrap both idx rows r-major into [16, 64] (positions 0-511 =
                # r0's batch, 512-1023 = r1's), then 4 transposed gathers of
                # 256 rows each; chunks 0/1 cover r0, 2/3 cover r1. Emission
                # order 0,2,1,3 so m-tiles 0-1 (chunks 0+2) unblock first.
                hsl = slice(hf * HB, (hf + 1) * HB)
                stg = spool.tile([32, 64], I16, tag="stage")
                stg_j = stg[0:32, :].rearrange("p (j hq) -> p j hq", j=2)
                with nc.allow_non_contiguous_dma(reason="tiny idx wrap"):
                    for r in range(2):
                        nc.scalar.dma_start(
                            stg[0:32, 32 * r : 32 * r + 16],
                            idx16[r : r + 1, hsl].rearrange(
                                "p (a b) -> p a b", b=16
                            ),
                        )
                nc.vector.tensor_copy(stg_j[:, :, 16:32], stg_j[:, :, 0:16])
                idxw = spool.tile([128, 64], I16, tag="idxw")
                for g in range(4):
                    nc.vector.transpose(idxw[32 * g : 32 * (g + 1), :], stg[:])
                for c in (0, 2, 1, 3):
                    g8 = spool.tile(
                        [128, 8, 256], F8, tag=f"g{hf}{c}", bufs=1
                    )
                    nc.gpsimd.dma_gather(
                        out_ap=g8[:],
                        in_ap=mem_t[:],
                        idxs_ap=idxw[:, 16 * c : 16 * (c + 1)],
                        num_idxs=256,
                        num_idxs_reg=256,
                        elem_size=H,
                        transpose=True,
                    )
                    g_tiles[hf][c] = g8[:].rearrange(
                        "p (c t) i -> p c (t i)", t=2
                    )

            # ---------------- main matmuls + epilogue ----------------
            ps_tiles = {}

            def emit_xh(m, si):
                # si=0: x-part (opens the tile's accumulation); si=1: h-part
                if si == 0:
                    ps = pp_main.tile([128, H], F32, tag="acc", name=f"acc{m}")
                    ps_tiles[m] = ps
                else:
                    ps = ps_tiles[m]
                act = (x8_sb, h8_sb)[si]
                ms = slice(m * 128, (m + 1) * 128)
                for kp in range(KC // 2):
                    lhsT = act[:, 2 * kp : 2 * kp + 2, ms]
                    for n in range(2):
                        nc.tensor.matmul(
                            ps[:, n * 512 : (n + 1) * 512],
                            lhsT,
                            wxu_sb[si][:, 2 * kp : 2 * kp + 2,
                                       n * 512 : (n + 1) * 512],
                            start=(si == 0 and kp == 0),
                            stop=False,
                            perf_mode=DR,
                        )

            def emit_mem_epilogue(m):
                ps = ps_tiles.pop(m)
                ms = slice(m * 128, (m + 1) * 128)
                mm = m % 4
                for si in range(2):
                    g_v = g_tiles[m // 4][2 * si + mm // 2]
                    off = 256 * (mm % 2)
                    for c in range(4):
                        lhsT = g_v[:, c, off : off + 256]
                        for n in range(2):
                            nc.tensor.matmul(
                                ps[:, n * 512 : (n + 1) * 512],
                                lhsT,
                                wq_sb[si][:, c, :, n * 512 : (n + 1) * 512],
                                start=False,
                                stop=(si == 1 and c == 3),
                                perf_mode=DRI,
                            )

                # t = pre + bias (bias varies along the free/feature dim)
                t = epool.tile([128, H], BF16, tag="t")
                nc.vector.tensor_tensor(
                    t[:], ps[:], bias_sb[:], mybir.AluOpType.add
                )
                # layernorm stats
                st6 = epool.tile([128, 2, 6], F32, tag="st6")
                for a in range(2):
                    nc.vector.bn_stats(st6[:, a, :], t[:, a * 512 : (a + 1) * 512])
                mv = epool.tile([128, 2], F32, tag="mv")
                nc.vector.bn_aggr(mv[:], st6.rearrange("p a b -> p (a b)"))
                # rstd = 1/sqrt(var + eps) via quake bit trick + 1 Newton
                # (max rel err ~1.8e-3; all on DVE, no ACT table swap)
                sc = epool.tile([128, 4], F32, tag="sc")
                v = sc[:, 0:1]
                nc.vector.tensor_scalar(
                    v, mv[:, 1:2], eps_sb[:, 0:1], None, mybir.AluOpType.add
                )
                y0i = sc[:, 1:2].bitcast(I32)
                nc.vector.tensor_scalar(
                    y0i, v.bitcast(I32), 1, None,
                    mybir.AluOpType.logical_shift_right,
                )
                nc.vector.tensor_scalar(
                    y0i, y0i, -1, QUAKE,
                    mybir.AluOpType.mult, mybir.AluOpType.add,
                )
                y0 = sc[:, 1:2]
                a_t = sc[:, 2:3]
                nc.vector.tensor_tensor(a_t, y0, y0, mybir.AluOpType.mult)
                nc.vector.tensor_tensor(a_t, a_t, v, mybir.AluOpType.mult)
                nc.vector.tensor_scalar(
                    a_t, a_t, -0.5, 1.5, mybir.AluOpType.mult, mybir.AluOpType.add
                )
                rstd = sc[:, 3:4]
                nc.vector.tensor_tensor(rstd, y0, a_t, mybir.AluOpType.mult)
                # nmu = -mu * rstd
                nmu = sc[:, 1:2]
                nc.vector.tensor_scalar(
                    nmu, mv[:, 0:1], rstd, -1.0,
                    mybir.AluOpType.mult, mybir.AluOpType.mult,
                )
                o = epool.tile([128, H], BF16, tag="o")
                if trivial_gb:
                    nc.scalar.activation(
                        o[:], t[:], mybir.ActivationFunctionType.Sigmoid,
                        bias=nmu, scale=rstd,
                    )
                else:
                    xh = epool.tile([128, H], F32, tag="xh")
                    nc.scalar.activation(
                        xh[:], t[:], mybir.ActivationFunctionType.Identity,
                        bias=nmu, scale=rstd,
                    )
                    nc.vector.tensor_tensor(
                        xh[:], xh[:], gam_sb[:], mybir.AluOpType.mult
                    )
                    nc.vector.tensor_tensor(
                        xh[:], xh[:], bet_sb[:], mybir.AluOpType.add
                    )
                    nc.scalar.activation(
                        o[:], xh[:], mybir.ActivationFunctionType.Sigmoid,
                        bias=eps_sb[:, 1:2],
                    )
                nc.sync.dma_start(out_t[ms, :], o[:])

            emit_logits_half(0)
            emit_gather_half(0)
            emit_xh(0, 0)
            emit_xh(1, 0)
            emit_xh(2, 0)
            emit_xh(0, 1)
            emit_xh(1, 1)
            emit_logits_half(1)
            emit_gather_half(1)
            emit_xh(2, 1)
            emit_mem_epilogue(0)
            for m in range(3, MT - 1):
                emit_xh(m, 0)
                emit_xh(m, 1)
                emit_mem_epilogue(m - 2)
            # tail: spread the last three mem tiles between the final xh
            # parts so their epilogues overlap the remaining matmuls
            emit_xh(MT - 1, 0)
            emit_mem_epilogue(MT - 3)
            emit_xh(MT - 1, 1)
            emit_mem_epilogue(MT - 2)
            emit_mem_epilogue(MT - 1)

    nc.compile()
    return nc


def _to_kxp(a, dtype):
    """[batch, feat] -> [128, KC, batch] with feat = k*128 + p."""
    t = np.ascontiguousarray(a.T.reshape(KC, 128, -1).transpose(1, 0, 2))
    return t.astype(dtype)


def _rev_blocks(a):
    """Reverse each 128-row block along the batch dim of [batch, feat]."""
    return np.ascontiguousarray(
        a.reshape(-1, 128, a.shape[-1])[:, ::-1, :].reshape(a.shape)
    )


def prep(inputs):
    """Host-side shard/layout prep. Returns (in_maps, trivial_gb)."""
    x = np.asarray(inputs["x"], np.float32)
    h = np.asarray(inputs["h_prev"], np.float32)
    memory = np.asarray(inputs["memory"], np.float32)
    gamma = np.asarray(inputs["gamma"], np.float32)
    beta = np.asarray(inputs["beta"], np.float32)
    trivial_gb = bool(np.all(gamma == 1.0) and np.all(beta == 0.0))

    # W/U: w[p, k, n] = W[n, 128k+p] * S in fp8
    wxu = np.stack(
        [
            _to_kxp(np.asarray(inputs[n], np.float32) * WSCALE, f8np)
            for n in ("Ww", "Uw")
        ]
    )
    # Qr/Ql: w[p, c, b, n] = Q[n, 2*(128c+p)+b] * S in fp8
    wq = np.zeros((2, 128, 4, 2, H), f8np)
    qsum = np.zeros(H, np.float32)
    for s, name in enumerate(("Qrw", "Qlw")):
        q = np.asarray(inputs[name], np.float32) * WSCALE  # [out, in]
        q8 = q.astype(f8np)
        qsum += q8.astype(np.float32).sum(axis=1)
        q8v = q8.reshape(H, 4, 128, 2)  # [n, c, p, b]
        wq[s] = np.ascontiguousarray(q8v.transpose(2, 1, 3, 0))

    # Mw split into bf16 high/low pairs packed as f32 columns
    mw = _to_kxp(np.asarray(inputs["Mw"], np.float32), np.float32)  # [128,KC,28]
    mwb = mw.astype(bfnp)
    mws = (mw - mwb.astype(np.float32)).astype(bfnp)
    mwhl = np.stack([mwb, mws], axis=-1)  # [128, KC, 28, 2] bf16

    pw2 = np.zeros((2 * NB, 2), np.float32)
    pw2[:NB, 0] = 2.0 ** np.arange(NB - 1, -1, -1)
    pw2[NB:, 1] = pw2[:NB, 0]
    clip = np.array([[0.0, MEM // 2 - 1], [MEM // 2, MEM - 1]], np.float32)

    mem8 = (memory - 0.5).astype(f8np)
    bias = (
        np.asarray(inputs["Wb"], np.float32)
        + np.asarray(inputs["Ub"], np.float32)
        + np.asarray(inputs["Qrb"], np.float32)
        + np.asarray(inputs["Qlb"], np.float32)
    ) * WSCALE + 0.5 * qsum

    constc = np.zeros((128, 228), np.float32)
    constc[:, 0:224] = mwhl.reshape(128, 448).view(np.float32)
    constc[:2, 224:226] = clip
    constc[: 2 * NB, 226:227] = -np.asarray(inputs["Mb"], np.float32).reshape(
        2 * NB, 1
    )
    constc[: 2 * NB, 227:228] = pw2.astype(bfnp).view(np.float32)[:, 0:1]
    bias128 = np.ascontiguousarray(np.broadcast_to(bias, (128, H)), np.float32)

    common = dict(
        wxu_t=wxu, wq_t=wq, constc_t=constc, bias_t=bias128, mem_t=mem8
    )
    if not trivial_gb:
        common["gam_t"] = np.ascontiguousarray(np.broadcast_to(gamma, (128, H)))
        common["bet_t"] = np.ascontiguousarray(np.broadcast_to(beta, (128, H)))

    in_maps = []
    for c in range(NCORES):
        xs = x[c * BL : (c + 1) * BL]
        hs = h[c * BL : (c + 1) * BL]
        m = dict(
            x8_t=_to_kxp(_rev_blocks(xs), f8np),
            h8_t=_to_kxp(_rev_blocks(hs), f8np),
            **common,
        )
        for i in range(2):
            hh = hs[i * HB : (i + 1) * HB]
            hh16 = hh.astype(bfnp).astype(np.float32)
            m[f"h16{i}_t"] = _to_kxp(hh, bfnp)  # [128, KC, HB]
            m[f"r16{i}_t"] = _to_kxp(hh - hh16, bfnp)  # residual, bf16
        in_maps.append(m)
    return in_maps, trivial_gb


def get_nc(trivial_gb):
    key = ("nc", trivial_gb)
    if key not in _CACHE:
        _CACHE[key] = _build(trivial_gb)
    return _CACHE[key]


def run(inputs, trace=False, **kw):
    in_maps, trivial_gb = prep(inputs)
    nc = get_nc(trivial_gb)
    res = run_bass_kernel_spmd(
        nc, in_maps, core_ids=list(range(NCORES)), trace=trace, **kw
    )
    outs = []
    for c in range(NCORES):
        o = np.asarray(res.results[c]["out_t"]).astype(np.float32)
        outs.append(o.reshape(MT, 128, H)[:, ::-1, :].reshape(BL, H))
    return np.concatenate(outs, axis=0), res


def kernel(**inputs):
    return run(inputs)[0]
